# revision 1
# baseline (speedup 1.0000x reference)
"""DCN (DLRM-style deep & cross network) Trainium2 Bass kernel.

Sharding: data-parallel over batch across 8 NeuronCores (2048 samples/core).
Embedding tables + MLP weights are replicated to every core's HBM.

Per-core pipeline (activations kept feature-major, i.e. transposed, for PE):
  1. Embedding gather via InstDMAGatherAnt (dma_gather): one instruction per
     (category, batch-half), fetching quad-rows (4 vocab rows = 512B, the
     minimum 256B-aligned element) with int16 indices v//4 (< 25000). A
     copy + 3x copy_predicated chain then selects the right 32-float
     quarter per lookup into the combined feature tile.
  2. PE 128x128 transposes -> ct[k] = combined^T chunks [128, 512-batch]
  3. MLP: h^T = relu(W^T @ x^T) chains, bf16 matmuls with fp32 accumulate
  4. CrossNet: x_{l+1} = x_l (1+s_l) + b_l with s_l = alpha_l . x_l is
     affine in (x0, b0..b2) with per-sample scalar coefficients, so the
     whole cross stack + final Wc_x dot reduces to 4 packed dot products
     against x0 (alpha0..2, Wc_x) plus a tiny scalar chain using
     host-precomputed constants alpha_l.b_j and Wc_x.b_j.
  5. final = sigmoid(x-part + Wc_h . h3 + bc), assembled batch-natural.

Combined features are stored per (batch-half, 128-feature block) so each
feature block (= 4 categories) becomes consumable as soon as its 4
category gathers complete, letting PE transposes and MLP overlap the
gather pipeline.
"""

import numpy as np

import concourse.bass as bass
import concourse.mybir as mybir
import concourse.tile as tile
from concourse import bacc
from concourse.bass import broadcast_tensor_aps
from concourse.bass_utils import run_bass_kernel_spmd
from concourse.masks import make_identity

F32 = mybir.dt.float32
F32R = mybir.dt.float32r
BF16 = mybir.dt.bfloat16
I32 = mybir.dt.int32
I16 = mybir.dt.int16

B = 16384
NCORES = 8
BC = B // NCORES            # 2048 samples per core
NCAT = 26
VOCAB = 100000
EMB = 32
NNUM = 13
D = NCAT * EMB + NNUM       # 845
DEMB = NCAT * EMB           # 832

L1, L2, L3 = 1024, 512, 256
NCROSS = 3
KC = 7                      # feature chunks of 128 (6*128 + 77)
KW = [128] * 6 + [D - 6 * 128]
CPB = 128 // EMB            # 4 categories per 128-feature block
M1, M2, M3 = L1 // 128, L2 // 128, L3 // 128   # 8, 4, 2
NTILE = BC // 128           # 16 batch tiles per core
CHUNK = 512                 # batch chunk (matmul N)
NCHUNK = BC // CHUNK        # 4
TPC = CHUNK // 128          # 4 batch tiles per chunk

NHALF = 2                   # gather granularity: batch halves of 1024
HB = BC // NHALF            # 1024 lookups per (category, half)
TPH = NTILE // NHALF        # 8 tiles per half


def _build(cross_consts) -> bass.Bass:
    # cross_consts = (c10, c20, c21, d0, d1, d2):
    #   c_lj = alpha_l . cross_bias_j,  d_j = Wc_x . cross_bias_j
    c10, c20, c21, d0, d1, d2 = cross_consts
    WDT = BF16

    def _r(ap):
        return ap

    nc = bacc.Bacc("TRN2", target_bir_lowering=False, num_swdge_queues=2)

    d_emb = nc.dram_tensor("emb", [NCAT * VOCAB, EMB], F32, kind="ExternalInput")
    d_cat = nc.dram_tensor("cat", [128, NTILE * NCAT], I32, kind="ExternalInput")
    d_gidx = nc.dram_tensor(
        "gidx", [128, NHALF * NCAT * (HB // 16)], I16, kind="ExternalInput"
    )
    d_num = nc.dram_tensor("num", [128, NTILE * NNUM], F32, kind="ExternalInput")
    d_w1 = nc.dram_tensor("w1", [D, L1], WDT, kind="ExternalInput")
    d_w2 = nc.dram_tensor("w2", [L1, L2], WDT, kind="ExternalInput")
    d_w3 = nc.dram_tensor("w3", [L2, L3], WDT, kind="ExternalInput")
    d_b1 = nc.dram_tensor("b1r", [128, M1], F32, kind="ExternalInput")
    d_b2 = nc.dram_tensor("b2r", [128, M2], F32, kind="ExternalInput")
    d_b3 = nc.dram_tensor("b3r", [128, M3], F32, kind="ExternalInput")
    d_bc = nc.dram_tensor("bcr", [128, 1], F32, kind="ExternalInput")
    # avec: per k-chunk 4 columns [alpha0, alpha1, alpha2, wc_x]
    d_avec = nc.dram_tensor("avec", [128, KC * 4], WDT, kind="ExternalInput")
    d_wch = nc.dram_tensor("wch", [128, 2], WDT, kind="ExternalInput")
    d_out = nc.dram_tensor("out", [128, NTILE], F32, kind="ExternalOutput")

    with tile.TileContext(nc) as tc:
        with (
            tc.tile_pool(name="consts", bufs=1) as consts,
            tc.tile_pool(name="quadp", bufs=8) as quadp,
            tc.tile_pool(name="ctp", bufs=6) as ctp,
            tc.tile_pool(name="actp", bufs=3) as actp,
            tc.tile_pool(name="rowp", bufs=1) as rowp,
            tc.tile_pool(name="ps_mm", bufs=3, space="PSUM") as ps_mm,
            tc.tile_pool(name="ps_tp", bufs=3, space="PSUM") as ps_tp,
            tc.tile_pool(name="ps_a", bufs=1, space="PSUM") as ps_a,
            tc.tile_pool(name="ps_h", bufs=1, space="PSUM") as ps_h,
        ):
            # ---------------- constants / weights ----------------
            ident = consts.tile([128, 128], F32, name="ident")
            make_identity(nc, ident)
            ident_bf = consts.tile([128, 128], BF16, name="ident_bf")
            make_identity(nc, ident_bf)
            # Warm-up transpose: absorbs the identity-ready (Pool) wait into
            # PE's clock once.
            warm = ps_tp.tile([128, 4], F32, name="warm", tag="pst")
            nc.tensor.transpose(warm[0:4, 0:4], ident[0:4, 0:4], ident[0:4, 0:4])

            gidx = consts.tile_from(d_gidx[:], name="gidx_sb")
            cat_sb = consts.tile([128, NTILE * NCAT], I32, name="cat_sb")
            nc.sync.dma_start(cat_sb[:], d_cat[:])
            num_sb = consts.tile([128, NTILE * NNUM], F32, name="num_sb")
            nc.sync.dma_start(num_sb[:], d_num[:])
            avec = consts.tile_from(d_avec[:], name="avec_sb")
            wch = consts.tile_from(d_wch[:], name="wch_sb")

            # quarter-select masks: m_i = ((cat & 3) == i), i in 1..3
            qq = consts.tile([128, NTILE * NCAT], I32, name="qq")
            nc.vector.tensor_single_scalar(
                qq[:], cat_sb[:], 3, mybir.AluOpType.bitwise_and
            )
            masks = []
            for i in range(1, 4):
                mi = consts.tile([128, NTILE * NCAT], I32, name=f"m{i}")
                nc.vector.tensor_single_scalar(
                    mi[:], qq[:], i, mybir.AluOpType.is_equal
                )
                masks.append(mi[:].rearrange("p (T c) -> p T c", c=NCAT))


            # combined features, batch-natural, split per (half, feature
            # block) so downstream deps are tile-granular: block k holds
            # categories 4k..4k+3 (block 6: cats 24,25 + numericals)
            cnb = [
                [
                    consts.tile([128, TPH, KW[k]], BF16, name=f"cnb{h}_{k}")
                    for k in range(KC)
                ]
                for h in range(NHALF)
            ]

            # natural-layout accumulators for the final combine
            a_nat = consts.tile([128, NTILE * 4], F32, name="a_nat")
            h_nat = consts.tile([128, NTILE], F32, name="h_nat")
            out_nat = consts.tile([128, NTILE], F32, name="out_nat")

            # ---------------- gather + transpose + per-chunk compute -------
            ct_tiles = {}
            psa_tiles = {}

            def emit_transposes(h, k):
                kw = KW[k]
                for ci in (2 * h, 2 * h + 1):
                    ctk = ctp.tile([128, CHUNK], BF16, name=f"ct{k}", tag=f"ct{k}")
                    ct_tiles[(ci, k)] = ctk
                    for t in range(TPC):
                        tt = (ci - 2 * h) * TPC + t
                        pst = ps_tp.tile([128, 128], BF16, name="pst", tag="pst")
                        nc.tensor.transpose(
                            pst[0:kw, :],
                            cnb[h][k][:, tt, 0:kw],
                            ident_bf[:],
                        )
                        nc.any.tensor_copy(
                            ctk[0:kw, t * 128 : (t + 1) * 128], pst[0:kw, :]
                        )

            def emit_gather_half(h):
                # numerical features into block 6 cols 64:77 (ready early)
                for tt in range(TPH):
                    T = h * TPH + tt
                    nc.any.tensor_copy(
                        cnb[h][6][:, tt, 2 * EMB : KW[6]],
                        num_sb[:, T * NNUM : (T + 1) * NNUM],
                    )
                for c in range(NCAT):
                    quad = quadp.tile([128, TPH, 4 * EMB], F32, name="quad")
                    nc.gpsimd.dma_gather(
                        out_ap=quad[:],
                        in_ap=d_emb[c * VOCAB : (c + 1) * VOCAB, :].rearrange(
                            "(r q) e -> r (q e)", q=4
                        ),
                        idxs_ap=gidx[
                            :,
                            (h * NCAT + c) * (HB // 16) : (h * NCAT + c + 1)
                            * (HB // 16),
                        ],
                        num_idxs=HB,
                        num_idxs_reg=HB,
                        elem_size=4 * EMB,
                        queue_num=c % 2,
                    )
                    co = (c % CPB) * EMB
                    dest = cnb[h][c // CPB][:, :, co : co + EMB]
                    nc.scalar.copy(dest, quad[:, :, 0:EMB])
                    for i in range(1, 4):
                        mslice = masks[i - 1][:, h * TPH : (h + 1) * TPH, c : c + 1]
                        mb, _ = broadcast_tensor_aps(mslice, dest)
                        nc.vector.copy_predicated(
                            dest, mb, quad[:, :, i * EMB : (i + 1) * EMB]
                        )
                    if c % CPB == CPB - 1 and c // CPB < 6:
                        emit_transposes(h, c // CPB)
                    if c == NCAT - 1:
                        emit_transposes(h, 6)

            def emit_chunk(ci):
                ct = [ct_tiles[(ci, k)] for k in range(KC)]
                # cross-net dot products: [alpha0, alpha1, alpha2, wc_x]
                psa = ps_a.tile([4, CHUNK], F32, name="psa", tag="psa")
                for k in range(KC):
                    kw = KW[k]
                    nc.tensor.matmul(
                        psa[:],
                        avec[0:kw, k * 4 : (k + 1) * 4],
                        ct[k][0:kw, :],
                        start=(k == 0),
                        stop=(k == KC - 1),
                    )
                a_sb = actp.tile([4, CHUNK], F32, name="a_sb")
                nc.any.tensor_copy(a_sb[:], psa[:])
                for t in range(TPC):
                    pta = ps_tp.tile([128, 4], F32, name="pta", tag="pst")
                    nc.tensor.transpose(
                        pta[:],
                        a_sb[:, t * 128 : (t + 1) * 128],
                        ident[0:4, 0:4],
                    )
                    T = ci * TPC + t
                    nc.vector.tensor_copy(a_nat[:, T * 4 : (T + 1) * 4], pta[:])

                # MLP
                h1 = []
                for m in range(M1):
                    psm = ps_mm.tile([128, CHUNK], F32, name="psm")
                    for k in range(KC):
                        kw = KW[k]
                        nc.tensor.matmul(
                            psm[:],
                            w1[k][0:kw, m * 128 : (m + 1) * 128],
                            ct[k][0:kw, :],
                            start=(k == 0),
                            stop=(k == KC - 1),
                        )
                    h = actp.tile([128, CHUNK], BF16, name=f"h1_{m}")
                    nc.scalar.activation(
                        h[:], psm[:], mybir.ActivationFunctionType.Relu,
                        bias=b1r[:, m : m + 1],
                    )
                    h1.append(h)
                h2 = []
                for m in range(M2):
                    psm = ps_mm.tile([128, CHUNK], F32, name="psm")
                    for k in range(M1):
                        nc.tensor.matmul(
                            psm[:],
                            w2[k][:, m * 128 : (m + 1) * 128],
                            h1[k][:],
                            start=(k == 0),
                            stop=(k == M1 - 1),
                        )
                    h = actp.tile([128, CHUNK], BF16, name=f"h2_{m}")
                    nc.scalar.activation(
                        h[:], psm[:], mybir.ActivationFunctionType.Relu,
                        bias=b2r[:, m : m + 1],
                    )
                    h2.append(h)
                h3 = []
                for m in range(M3):
                    psm = ps_mm.tile([128, CHUNK], F32, name="psm")
                    for k in range(M2):
                        nc.tensor.matmul(
                            psm[:],
                            w3[k][:, m * 128 : (m + 1) * 128],
                            h2[k][:],
                            start=(k == 0),
                            stop=(k == M2 - 1),
                        )
                    h = actp.tile([128, CHUNK], BF16, name=f"h3_{m}")
                    nc.scalar.activation(
                        h[:], psm[:], mybir.ActivationFunctionType.Identity,
                        bias=b3r[:, m : m + 1],
                    )
                    h3.append(h)

                # h3 . wc_h -> row -> batch-natural
                psh = ps_h.tile([1, CHUNK], F32, name="psh", tag="psrow")
                for j in range(M3):
                    nc.tensor.matmul(
                        psh[:], wch[:, j : j + 1], h3[j][:],
                        start=(j == 0), stop=(j == M3 - 1),
                    )
                h_sb = actp.tile([1, CHUNK], F32, name="h_sb")
                nc.any.tensor_copy(h_sb[:], psh[:])
                for t in range(TPC):
                    pth = ps_tp.tile([128, 1], F32, name="pth", tag="pst")
                    nc.tensor.transpose(
                        pth[:], h_sb[:, t * 128 : (t + 1) * 128], ident[0:1, 0:1]
                    )
                    T = ci * TPC + t
                    nc.vector.tensor_copy(h_nat[:, T : T + 1], pth[:])

            # ------------- final combine (batch-natural, per half) ---------
            # x3 = p3*x0 + q30*b0 + q31*b1 + b2 with per-sample scalars from
            # the a-dots; Wc_x.x3 folds to p3*awc + q30*d0 + q31*d1 + d2.
            def emit_combine(h):
                NT = TPH  # tiles in this half
                av = a_nat[:, h * TPH * 4 : (h + 1) * TPH * 4].rearrange(
                    "p (t l) -> p t l", l=4
                )
                a0, a1, a2, awc = (av[:, :, l] for l in range(4))
                hn = h_nat[:, h * TPH : (h + 1) * TPH]

                def rtile(name):
                    return rowp.tile([128, NT], F32, name=name, tag=f"{name}_{h}")

                p1 = rtile("p1")            # 1 + s0
                nc.vector.tensor_scalar_add(p1[:], a0, 1.0)
                s1 = rtile("s1")            # s1 = p1*a1 (+ c10)
                nc.vector.tensor_mul(s1[:], a1, p1[:])
                if c10 != 0.0:
                    nc.vector.tensor_scalar_add(s1[:], s1[:], float(c10))
                u1 = rtile("u1")            # 1 + s1  (= q20)
                nc.vector.tensor_scalar_add(u1[:], s1[:], 1.0)
                p2 = rtile("p2")
                nc.vector.tensor_mul(p2[:], p1[:], u1[:])
                s2 = rtile("s2")            # s2 = p2*a2 + u1*c20 + c21
                nc.vector.tensor_mul(s2[:], a2, p2[:])
                if c20 != 0.0:
                    v20 = rtile("v20")
                    nc.vector.tensor_scalar_mul(v20[:], u1[:], float(c20))
                    nc.vector.tensor_add(s2[:], s2[:], v20[:])
                if c21 != 0.0:
                    nc.vector.tensor_scalar_add(s2[:], s2[:], float(c21))
                u2 = rtile("u2")            # 1 + s2
                nc.vector.tensor_scalar_add(u2[:], s2[:], 1.0)
                p3 = rtile("p3")
                nc.vector.tensor_mul(p3[:], p2[:], u2[:])
                fin = rtile("fin")          # awc*p3 (+ bias-derived terms)
                nc.vector.tensor_mul(fin[:], awc, p3[:])
                if d0 != 0.0:
                    q30 = rtile("q30")
                    nc.vector.tensor_mul(q30[:], u1[:], u2[:])
                    nc.vector.tensor_scalar_mul(q30[:], q30[:], float(d0))
                    nc.vector.tensor_add(fin[:], fin[:], q30[:])
                if d1 != 0.0:
                    w1t = rtile("w1t")
                    nc.vector.tensor_scalar_mul(w1t[:], u2[:], float(d1))
                    nc.vector.tensor_add(fin[:], fin[:], w1t[:])
                if d2 != 0.0:
                    nc.vector.tensor_scalar_add(fin[:], fin[:], float(d2))
                nc.vector.tensor_add(fin[:], fin[:], hn)
                ons = out_nat[:, h * TPH : (h + 1) * TPH]
                nc.scalar.activation(
                    ons, fin[:], mybir.ActivationFunctionType.Sigmoid,
                    bias=bcr[:, 0:1],
                )
                nc.sync.dma_start(d_out[:, h * TPH : (h + 1) * TPH], ons)


            emit_gather_half(0)
            w1 = [
                consts.tile_from(d_w1[k * 128 : k * 128 + KW[k], :], name=f"w1_{k}")
                for k in range(KC)
            ]
            w2 = [
                consts.tile_from(d_w2[k * 128 : (k + 1) * 128, :], name=f"w2_{k}")
                for k in range(M1)
            ]
            w3 = [
                consts.tile_from(d_w3[k * 128 : (k + 1) * 128, :], name=f"w3_{k}")
                for k in range(M2)
            ]
            b1r = consts.tile_from(d_b1[:], name="b1r_sb")
            b2r = consts.tile_from(d_b2[:], name="b2r_sb")
            b3r = consts.tile_from(d_b3[:], name="b3r_sb")
            bcr = consts.tile_from(d_bc[:], name="bcr_sb")
            emit_chunk(0)
            emit_chunk(1)
            emit_combine(0)
            emit_gather_half(1)
            emit_chunk(2)
            emit_chunk(3)

            emit_combine(1)

    nc.compile()
    return nc


_CACHE: dict = {}
DEBUG_DUMP = False


def _get_nc(cross_consts) -> bass.Bass:
    key = (cross_consts, DEBUG_DUMP)
    if key not in _CACHE:
        _CACHE[key] = _build(cross_consts)
    return _CACHE[key]


def kernel(
    categorical_input,
    numerical_input,
    emb_tables,
    alphas,
    cross_bias,
    W1, b1, W2, b2, W3, b3, Wc, bc,
) -> np.ndarray:
    cat = np.ascontiguousarray(np.asarray(categorical_input, dtype=np.int64))
    num = np.ascontiguousarray(np.asarray(numerical_input, dtype=np.float32))
    emb = np.ascontiguousarray(
        np.asarray(emb_tables, dtype=np.float32).reshape(NCAT * VOCAB, EMB)
    )
    alphas = np.asarray(alphas, dtype=np.float32)
    cross_bias = np.asarray(cross_bias, dtype=np.float32)
    W1 = np.ascontiguousarray(np.asarray(W1, dtype=np.float32))
    W2 = np.ascontiguousarray(np.asarray(W2, dtype=np.float32))
    W3 = np.ascontiguousarray(np.asarray(W3, dtype=np.float32))
    Wc = np.asarray(Wc, dtype=np.float32)
    b1 = np.asarray(b1, dtype=np.float32)
    b2 = np.asarray(b2, dtype=np.float32)
    b3 = np.asarray(b3, dtype=np.float32)
    bc = np.asarray(bc, dtype=np.float32)

    # host scalar constants folding cross_bias into the per-sample chain
    cross_consts = (
        float(np.dot(alphas[1], cross_bias[0])),
        float(np.dot(alphas[2], cross_bias[0])),
        float(np.dot(alphas[2], cross_bias[1])),
        float(np.dot(Wc[:D, 0], cross_bias[0])),
        float(np.dot(Wc[:D, 0], cross_bias[1])),
        float(np.dot(Wc[:D, 0], cross_bias[2])),
    )
    nc = _get_nc(cross_consts)

    def to_dev(v):  # [D(,k)] -> [KC*128(,k)] zero-padded
        shape = (KC * 128,) + v.shape[1:]
        p = np.zeros(shape, np.float32)
        p[:D] = v
        return p

    W1p = W1

    def pad_col(v):  # [845] -> [128, KC] column-chunked, zero-padded
        return to_dev(v).reshape(KC, 128).T.copy()

    avec = np.zeros((128, KC * 4), np.float32)
    for l in range(NCROSS):
        avec[:, l::4] = pad_col(alphas[l])
    avec[:, 3::4] = pad_col(Wc[:D, 0])
    wch = Wc[D : D + L3, 0].reshape(2, 128).T.copy()
    b1r = b1.reshape(M1, 128).T.copy()
    b2r = b2.reshape(M2, 128).T.copy()
    b3r = b3.reshape(M3, 128).T.copy()
    bcr = np.broadcast_to(bc.reshape(1, 1), (128, 1)).copy()

    import ml_dtypes

    bf = ml_dtypes.bfloat16
    common = {
        "emb": emb,
        "w1": W1p.astype(bf),
        "w2": W2.astype(bf),
        "w3": W3.astype(bf),
        "b1r": b1r,
        "b2r": b2r,
        "b3r": b3r,
        "bcr": bcr,
        "avec": avec.astype(bf),
        "wch": wch.astype(bf),
    }
    in_maps = []
    for core in range(NCORES):
        cs = cat[core * BC : (core + 1) * BC].astype(np.int32)  # [2048, 26]
        ns = num[core * BC : (core + 1) * BC]
        catr = np.ascontiguousarray(
            cs.reshape(NTILE, 128, NCAT).transpose(1, 0, 2).reshape(128, NTILE * NCAT)
        )
        numr = np.ascontiguousarray(
            ns.reshape(NTILE, 128, NNUM).transpose(1, 0, 2).reshape(128, NTILE * NNUM)
        )
        # gather indices: per (half, category) block of 64 cols, int16 v//4,
        # lookup i at [i % 16, i // 16]
        gi = np.zeros((128, NHALF * NCAT * (HB // 16)), np.int16)
        for h in range(NHALF):
            vs = cs[h * HB : (h + 1) * HB]  # [1024, 26]
            q4 = (vs // 4).astype(np.int16)  # [1024, 26]
            # [1024, 26] -> per cat [16, 64]: i -> (i%16, i//16)
            wrapped = q4.reshape(HB // 16, 16, NCAT).transpose(1, 0, 2)  # [16,64,26]
            for c in range(NCAT):
                blk = (h * NCAT + c) * (HB // 16)
                gi[0:16, blk : blk + HB // 16] = wrapped[:, :, c]
        for g in range(1, 8):
            gi[g * 16 : (g + 1) * 16] = gi[0:16]
        in_maps.append({**common, "cat": catr, "num": numr, "gidx": gi})

    res = run_bass_kernel_spmd(nc, in_maps, core_ids=list(range(NCORES)))
    outs = []
    for core in range(NCORES):
        o = res.results[core]["out"]  # [128, NTILE], sample T*128+p at [p, T]
        outs.append(o.T.reshape(BC, 1))
    return np.concatenate(outs, axis=0).astype(np.float32)



# revision 20
# speedup vs baseline: 1.5487x; 1.5487x over previous
"""DCN (DLRM-style deep & cross network) Trainium2 Bass kernel, v2.

Sharding: data-parallel over batch across 8 NeuronCores (2048 samples/core).
Embedding tables (bf16) + MLP weights (fp8) replicated to every core's HBM.

Per-core pipeline:
  1. Embedding gather via dma_gather: one instruction per (category,
     batch-half of 1024). Table stored bf16 so a 256B element = 4 vocab rows
     (int16 idx = v//4 < 25000); a copy + 3x copy_predicated select the right
     32-value quarter into batch-natural feature blocks (cnb).
  2. PE transposes feature blocks -> ct[ci] = [128, 7 kplanes, 512] fp8
     (feature-major), converted to fp8 during the PSUM->SBUF copy.
  3. MLP in fp8 with DoubleRow matmuls (2 k-planes per instruction):
     h1 = relu(W1^T x) with b1 folded into W1 via a constant-1 feature row
     (ct row 845), so relu is a pure max; W2/W3 biases fused into the
     activation op (Act) or a dual-op tensor_scalar (DVE).
  4. CrossNet reduces to 4 packed dot products (alpha0..2, Wc_x) against x0
     plus a tiny per-sample scalar chain using host-folded constants.
  5. out = sigmoid(x-part + Wc_h . h3 + bc), assembled batch-natural.

Emission is time-ordered: half-0 gathers stream selects/transposes; half-1
gathers interleave with half-0's MLP stages so every engine's in-order queue
matches expected data-arrival times.
"""

import numpy as np

import concourse.bass as bass
import concourse.mybir as mybir
import concourse.tile as tile
from concourse import bacc
from concourse.bass import broadcast_tensor_aps
from concourse.bass_utils import run_bass_kernel_spmd
from concourse.masks import make_identity

F32 = mybir.dt.float32
BF16 = mybir.dt.bfloat16
FP8 = mybir.dt.float8e4
I32 = mybir.dt.int32
I16 = mybir.dt.int16

DR = mybir.MatmulPerfMode.DoubleRow
AF = mybir.ActivationFunctionType
ALU = mybir.AluOpType

B = 16384
NCORES = 8
BC = B // NCORES            # 2048 samples per core
NCAT = 26
VOCAB = 100000
EMB = 32
NNUM = 13
D = NCAT * EMB + NNUM       # 845
KP = 7                      # feature k-planes of 128 (845 -> 896 padded)
KPAD = 8                    # ct planes incl. zero pad plane for DoubleRow
BIAS_ROW = D                # ct row 845 == 1.0 (W1 bias fold)

L1, L2, L3 = 1024, 512, 256
M1, M2, M3 = L1 // 128, L2 // 128, L3 // 128   # 8, 4, 2
P1 = M1 // 2                # W1 output m-pairs: 4
KP2, KP3 = M1 // 2, M2 // 2                    # contraction pairs: W2=4, W3=2

NTILE = BC // 128           # 16 batch tiles per core
CHUNK = 512
NCHUNK = BC // CHUNK        # 4
TPC = CHUNK // 128          # 4

NHALF = 2
HB = BC // NHALF            # 1024 lookups per (category, half)
TPH = NTILE // NHALF        # 8 tiles per half
CPH = TPH // TPC            # 2 chunks per half

NQUAD = VOCAB // 4          # 25000 quad-rows per category
EARLY = 2                   # cats with early-loaded gather indices (half 0)


def _build(cross_consts) -> bass.Bass:
    c10, c20, c21, d0, d1, d2 = cross_consts

    nc = bacc.Bacc("TRN2", target_bir_lowering=False, num_swdge_queues=2)

    d_emb = nc.dram_tensor("emb", [NCAT * NQUAD, 128], BF16, kind="ExternalInput")
    d_gidx_a0 = nc.dram_tensor("gidx_a0", [128, EARLY * (HB // 16)], I16,
                               kind="ExternalInput")
    d_gidx_a1 = nc.dram_tensor("gidx_a1", [128, (NCAT - EARLY) * (HB // 16)],
                               I16, kind="ExternalInput")
    d_gidx_b = nc.dram_tensor("gidx_b", [128, NCAT * (HB // 16)], I16,
                              kind="ExternalInput")
    d_cat = nc.dram_tensor("cat", [128, NTILE * NCAT], I32, kind="ExternalInput")
    d_num = nc.dram_tensor("num", [128, NTILE * NNUM], F32, kind="ExternalInput")
    # W1 padded to [1024, 1024] with row 845 = b1: 4 DR pair-tiles
    d_w1 = nc.dram_tensor("w1", [128, 4 * 2 * L1], FP8, kind="ExternalInput")
    d_w2 = nc.dram_tensor("w2", [128, KP2 * 2 * L2], FP8, kind="ExternalInput")
    d_w3 = nc.dram_tensor("w3", [128, KP3 * 2 * L3], FP8, kind="ExternalInput")
    d_avec = nc.dram_tensor("avec", [128, KPAD * 128], FP8, kind="ExternalInput")
    d_wch = nc.dram_tensor("wch", [128, 2 * 128], FP8, kind="ExternalInput")
    d_b2 = nc.dram_tensor("b2r", [128, M2], F32, kind="ExternalInput")
    d_b3 = nc.dram_tensor("b3r", [128, M3], F32, kind="ExternalInput")
    d_bc = nc.dram_tensor("bcr", [128, 1], F32, kind="ExternalInput")
    d_out = nc.dram_tensor("out", [128, NTILE], F32, kind="ExternalOutput")

    with tile.TileContext(nc) as tc:
        with (
            tc.tile_pool(name="consts", bufs=1) as consts,
            tc.tile_pool(name="quadp", bufs=8) as quadp,
            tc.tile_pool(name="ctp", bufs=1) as ctp,
            tc.tile_pool(name="actp", bufs=2) as actp,
            tc.tile_pool(name="rowp", bufs=1) as rowp,
            tc.tile_pool(name="ps_w1", bufs=4, space="PSUM") as ps_w1,
            tc.tile_pool(name="ps_mm", bufs=2, space="PSUM") as ps_mm,
            tc.tile_pool(name="ps_a", bufs=1, space="PSUM") as ps_a,
            tc.tile_pool(name="ps_tp", bufs=1, space="PSUM") as ps_tp,
        ):
            # ---------------- constants / inputs ----------------
            ident = consts.tile([128, 128], F32, name="ident")
            make_identity(nc, ident)
            ident_bf = consts.tile([128, 128], BF16, name="ident_bf")
            make_identity(nc, ident_bf)
            warm = ps_a.tile([128, 4], F32, name="warm", tag="psa")
            nc.tensor.transpose(warm[0:4, 0:4], ident[0:4, 0:4], ident[0:4, 0:4])
            sigw = consts.tile([1, 1], F32, name="sigw")
            nc.scalar.activation(sigw[:], ident[0:1, 0:1], AF.Sigmoid)

            gidx_a0 = consts.tile_from(d_gidx_a0[:], name="gidx_a0_sb")
            cat_sb = consts.tile([128, NTILE * NCAT], I32, name="cat_sb")
            nc.sync.dma_start(cat_sb[:], d_cat[:])
            num_sb = consts.tile([128, NTILE * NNUM], F32, name="num_sb")
            nc.sync.dma_start(num_sb[:], d_num[:])
            gidx_a1 = consts.tile_from(d_gidx_a1[:], name="gidx_a1_sb")
            gidx_b = consts.tile_from(d_gidx_b[:], name="gidx_b_sb")

            def gidx_slice(h, c):
                n = HB // 16
                if h == 1:
                    return gidx_b[:, c * n:(c + 1) * n]
                if c < EARLY:
                    return gidx_a0[:, c * n:(c + 1) * n]
                return gidx_a1[:, (c - EARLY) * n:(c - EARLY + 1) * n]

            # weights (fp8) + biases
            w1p = [
                consts.tile_from(d_w1[:, p * 2 * L1:(p + 1) * 2 * L1],
                                 name=f"w1p{p}").rearrange("p (j n) -> p j n", j=2)
                for p in range(4)
            ]
            w2p = [
                consts.tile_from(d_w2[:, p * 2 * L2:(p + 1) * 2 * L2],
                                 name=f"w2p{p}").rearrange("p (j n) -> p j n", j=2)
                for p in range(KP2)
            ]
            w3p = [
                consts.tile_from(d_w3[:, p * 2 * L3:(p + 1) * 2 * L3],
                                 name=f"w3p{p}").rearrange("p (j n) -> p j n", j=2)
                for p in range(KP3)
            ]
            avec = consts.tile_from(d_avec[:], name="avec_sb").rearrange(
                "p (k l) -> p k l", l=128
            )
            wch = consts.tile_from(d_wch[:], name="wch_sb").rearrange(
                "p (j o) -> p j o", o=128
            )
            b2r = consts.tile_from(d_b2[:], name="b2r_sb")
            b3r = consts.tile_from(d_b3[:], name="b3r_sb")
            bcr = consts.tile_from(d_bc[:], name="bcr_sb")

            # quarter-select masks: m_i = ((cat & 3) == i), i in 1..3
            qq = consts.tile([128, NTILE * NCAT], I32, name="qq")
            nc.vector.tensor_single_scalar(qq[:], cat_sb[:], 3, ALU.bitwise_and)
            masks = []
            for i in range(1, 4):
                mi = consts.tile([128, NTILE * NCAT], I32, name=f"m{i}")
                nc.vector.tensor_single_scalar(mi[:], qq[:], i, ALU.is_equal)
                masks.append(mi[:].rearrange("p (T c) -> p T c", c=NCAT))

            # batch-natural feature blocks, per (half, 128-feature block)
            cnb = [
                [
                    consts.tile([128, TPH, 128], BF16, name=f"cnb{h}_{k}")
                    for k in range(KP)
                ]
                for h in range(NHALF)
            ]
            numv = num_sb[:].rearrange("p (T u) -> p T u", u=NNUM)

            def emit_blk6_init(h):
                blk6 = cnb[h][6]
                nc.scalar.copy(blk6[:, :, 2 * EMB:2 * EMB + NNUM],
                               numv[:, h * TPH:(h + 1) * TPH, :])
                nc.vector.memset(blk6[:, :, 77:78], 1.0)   # W1 bias-fold row
                nc.vector.memset(blk6[:, :, 78:128], 0.0)

            # feature-major fp8 activations per chunk
            ct = [
                ctp.tile([128, KPAD, CHUNK], FP8, name=f"ct{ci}", tag=f"ct{ci}")
                for ci in range(NCHUNK)
            ]

            def emit_ct_zero():
                for ci in range(NCHUNK):
                    nc.gpsimd.memset(ct[ci][:, 7, :], 0.0)

            # natural-layout accumulators for the final combine
            a_nat = consts.tile([128, NTILE * 4], F32, name="a_nat")
            h_nat = consts.tile([128, NTILE], F32, name="h_nat")
            out_nat = consts.tile([128, NTILE], F32, name="out_nat")

            # ---------------- gather + select + transpose ----------------
            def emit_transposes(h, k):
                for cl in range(CPH):
                    ci = CPH * h + cl
                    pst = ps_tp.tile([128, CHUNK], BF16, name="pst", tag="pst")
                    for t in range(TPC):
                        tt = cl * TPC + t
                        nc.tensor.transpose(
                            pst[:, t * 128:(t + 1) * 128],
                            cnb[h][k][:, tt, :],
                            ident_bf[:],
                        )
                    nc.scalar.copy(ct[ci][:, k, :], pst[:])

            CAT_ORDER = [24, 25] + list(range(24))
            BLK_NEED = [4] * 6 + [2]

            def emit_gather(h, c, seq, blk_left, pool_pred=False):
                quad = quadp.tile([128, TPH, 128], BF16, name="quad", tag="quad")
                nc.gpsimd.dma_gather(
                    out_ap=quad[:],
                    in_ap=d_emb[c * NQUAD:(c + 1) * NQUAD, :],
                    idxs_ap=gidx_slice(h, c),
                    num_idxs=HB,
                    num_idxs_reg=HB,
                    elem_size=128,
                    queue_num=seq % 2,
                )
                co = (c % 4) * EMB
                dest = cnb[h][c // 4][:, :, co:co + EMB]
                nc.vector.tensor_copy(dest, quad[:, :, 0:EMB])
                for i in range(1, 4):
                    mslice = masks[i - 1][:, h * TPH:(h + 1) * TPH, c:c + 1]
                    mb, _ = broadcast_tensor_aps(mslice, dest)
                    nc.vector.copy_predicated(
                        dest, mb, quad[:, :, i * EMB:(i + 1) * EMB]
                    )
                blk = c // 4
                blk_left[blk] -= 1
                if blk_left[blk] == 0:
                    emit_transposes(h, blk)

            # ---------------- MLP + cross dots, per chunk ----------------
            def dr_chain(out_ap, pairs, rhs_pairs):
                """Accumulate DoubleRow pairs into out_ap."""
                n = len(pairs)
                for p, (lhs, rhs) in enumerate(zip(pairs, rhs_pairs)):
                    nc.tensor.matmul(out_ap, lhs, rhs, start=(p == 0),
                                     stop=(p == n - 1), perf_mode=DR)

            def pt_pool(t, tail, shape):
                if tail and t % 2 == 1:
                    return ps_tp.tile(shape, F32, name="ptx", tag="pst")
                return ps_a.tile(shape, F32, name="ptx", tag="psa")

            def stage_adots(ci, tail):
                cta = ct[ci]
                rp = [cta[:, 2 * p:2 * p + 2, :] for p in range(4)]
                psa = ps_a.tile([128, CHUNK], F32, name="psa", tag="psa")
                dr_chain(psa[:],
                         [avec[:, 2 * p:2 * p + 2, :] for p in range(4)], rp)
                a_sb = actp.tile([4, CHUNK], F32, name="a_sb", tag="a_sb")
                nc.vector.tensor_copy(a_sb[:], psa[0:4, :])
                for t in range(TPC):
                    pta = pt_pool(t, tail, [128, 4])
                    nc.tensor.transpose(
                        pta[:], a_sb[:, t * 128:(t + 1) * 128], ident[0:4, 0:4]
                    )
                    T = ci * TPC + t
                    nc.vector.tensor_copy(a_nat[:, T * 4:(T + 1) * 4], pta[:])

            def stage_w1_pair(ci, j, h1, tail):
                cta = ct[ci]
                rp = [cta[:, 2 * p:2 * p + 2, :] for p in range(4)]
                for mo in range(2):
                    m = 2 * j + mo
                    psm = w1_pool(m + ci, tail)
                    dr_chain(psm[:],
                             [w1p[p][:, :, m * 128:(m + 1) * 128]
                              for p in range(4)], rp)
                    dst = h1[:, m, :]
                    if not tail or (ci + m) % 2 == 0:
                        nc.scalar.activation(dst, psm[:], AF.Relu)
                    else:
                        nc.vector.tensor_scalar_max(dst, psm[:], 0.0)

            def mm_pool(m, tail):
                if tail and m % 2 == 1:
                    return ps_w1.tile([128, CHUNK], F32, name="psm", tag="psw1")
                return ps_mm.tile([128, CHUNK], F32, name="psm", tag="psm")

            def w1_pool(m, tail):
                if tail and m % 3 == 2:
                    return ps_mm.tile([128, CHUNK], F32, name="psm", tag="psm")
                return ps_w1.tile([128, CHUNK], F32, name="psw1", tag="psw1")

            def stage_w2_pair(ci, half_idx, h1, h2, tail):
                for mo in range(2):
                    m = 2 * half_idx + mo
                    psm = mm_pool(m + ci, tail)
                    for p in range(KP2):
                        nc.tensor.matmul(
                            psm[:], w2p[p][:, :, m * 128:(m + 1) * 128],
                            h1[:, 2 * p:2 * p + 2, :],
                            start=(p == 0), stop=(p == KP2 - 1), perf_mode=DR,
                        )
                    dst = h2[:, m, :]
                    if not tail or (ci + m) % 2 == 0:
                        nc.scalar.activation(dst, psm[:], AF.Relu,
                                             bias=b2r[:, m:m + 1])
                    else:
                        nc.vector.tensor_scalar(dst, psm[:], 0.0,
                                                b2r[:, m:m + 1], ALU.max,
                                                ALU.add)

            def stage_w3(ci, h2, h3, tail):
                for m in range(M3):
                    psm = mm_pool(m + ci, tail)
                    for p in range(KP3):
                        nc.tensor.matmul(
                            psm[:], w3p[p][:, :, m * 128:(m + 1) * 128],
                            h2[:, 2 * p:2 * p + 2, :],
                            start=(p == 0), stop=(p == KP3 - 1), perf_mode=DR,
                        )
                    dst = h3[:, m, :]
                    if not tail or m == 0:
                        nc.scalar.activation(dst, psm[:], AF.Identity,
                                             bias=b3r[:, m:m + 1])
                    else:
                        nc.vector.tensor_scalar_add(dst, psm[:], b3r[:, m:m + 1])

            def stage_wch(ci, h3, tail):
                psh = mm_pool(ci, tail)
                nc.tensor.matmul(psh[:], wch[:], h3[:], start=True,
                                 stop=True, perf_mode=DR)
                h_sb = actp.tile([1, CHUNK], F32, name="h_sb", tag="h_sb")
                nc.vector.tensor_copy(h_sb[:], psh[0:1, :])
                for t in range(TPC):
                    pth = pt_pool(t, tail, [128, 1])
                    nc.tensor.transpose(
                        pth[:], h_sb[:, t * 128:(t + 1) * 128], ident[0:1, 0:1]
                    )
                    T = ci * TPC + t
                    nc.vector.tensor_copy(h_nat[:, T:T + 1], pth[:])

            def chunk_stages(ci, tail):
                h1 = actp.tile([128, M1, CHUNK], FP8, name=f"h1_{ci}", tag="h1")
                h2 = actp.tile([128, M2, CHUNK], FP8, name=f"h2_{ci}", tag="h2")
                h3 = actp.tile([128, M3, CHUNK], FP8, name=f"h3_{ci}", tag="h3")
                out = [lambda: stage_adots(ci, tail)]
                for j in range(P1):
                    out.append(lambda j=j: stage_w1_pair(ci, j, h1, tail))
                for hx in range(2):
                    out.append(lambda hx=hx: stage_w2_pair(ci, hx, h1, h2, tail))
                out.append(lambda: stage_w3(ci, h2, h3, tail))
                out.append(lambda: stage_wch(ci, h3, tail))
                return out

            def mlp_stages(h, tail):
                """Interleaved two-chunk stage list for half h + combines."""
                ca, cb = CPH * h, CPH * h + 1
                a = chunk_stages(ca, tail)
                b = chunk_stages(cb, tail)
                out = []
                for x, y in zip(a, b):
                    out.append(x)
                    out.append(y)
                out.append(lambda: emit_combine(ca))
                out.append(lambda: emit_combine(cb))
                return out

            # ------------- final combine (batch-natural, per chunk) ---------
            def emit_combine(ci):
                NT = TPC
                t0 = ci * TPC
                av = a_nat[:, t0 * 4:(t0 + NT) * 4].rearrange(
                    "p (t l) -> p t l", l=4
                )
                a0, a1, a2, awc = (av[:, :, l] for l in range(4))
                hn = h_nat[:, t0:t0 + NT]

                def rtile(name):
                    return rowp.tile([128, NT], F32, name=name, tag=f"{name}_{ci}")

                p1 = rtile("p1")            # 1 + s0
                nc.vector.tensor_scalar_add(p1[:], a0, 1.0)
                s1 = rtile("s1")            # s1 = p1*a1 (+ c10)
                nc.vector.tensor_mul(s1[:], a1, p1[:])
                if c10 != 0.0:
                    nc.vector.tensor_scalar_add(s1[:], s1[:], float(c10))
                u1 = rtile("u1")            # 1 + s1
                nc.vector.tensor_scalar_add(u1[:], s1[:], 1.0)
                p2 = rtile("p2")
                nc.vector.tensor_mul(p2[:], p1[:], u1[:])
                s2 = rtile("s2")            # s2 = p2*a2 + u1*c20 + c21
                nc.vector.tensor_mul(s2[:], a2, p2[:])
                if c20 != 0.0:
                    v20 = rtile("v20")
                    nc.vector.tensor_scalar_mul(v20[:], u1[:], float(c20))
                    nc.vector.tensor_add(s2[:], s2[:], v20[:])
                if c21 != 0.0:
                    nc.vector.tensor_scalar_add(s2[:], s2[:], float(c21))
                u2 = rtile("u2")            # 1 + s2
                nc.vector.tensor_scalar_add(u2[:], s2[:], 1.0)
                p3 = rtile("p3")
                nc.vector.tensor_mul(p3[:], p2[:], u2[:])
                fin = rtile("fin")          # awc*p3 (+ bias-derived terms)
                nc.vector.tensor_mul(fin[:], awc, p3[:])
                if d0 != 0.0:
                    q30 = rtile("q30")
                    nc.vector.tensor_mul(q30[:], u1[:], u2[:])
                    nc.vector.tensor_scalar_mul(q30[:], q30[:], float(d0))
                    nc.vector.tensor_add(fin[:], fin[:], q30[:])
                if d1 != 0.0:
                    w1t = rtile("w1t")
                    nc.vector.tensor_scalar_mul(w1t[:], u2[:], float(d1))
                    nc.vector.tensor_add(fin[:], fin[:], w1t[:])
                if d2 != 0.0:
                    nc.vector.tensor_scalar_add(fin[:], fin[:], float(d2))
                nc.vector.tensor_add(fin[:], fin[:], hn)
                ons = out_nat[:, t0:t0 + NT]
                nc.scalar.activation(ons, fin[:], AF.Sigmoid, bias=bcr[:, 0:1])
                nc.scalar.dma_start(d_out[:, t0:t0 + NT], ons)

            # ---------------- time-ordered emission ----------------
            emit_blk6_init(0)
            emit_blk6_init(1)
            blk_left0 = list(BLK_NEED)
            for seq, c in enumerate(CAT_ORDER):
                emit_gather(0, c, seq, blk_left0)
                if seq == 2:
                    emit_ct_zero()
            stages0 = mlp_stages(0, tail=False)
            # interleave half-1 gathers with half-0 MLP stages
            si = 0
            blk_left1 = list(BLK_NEED)
            for seq, c in enumerate(CAT_ORDER):
                emit_gather(1, c, seq, blk_left1, pool_pred=(seq >= 21))
                take = ((seq + 1) * len(stages0)) // NCAT - si
                for _ in range(take):
                    stages0[si]()
                    si += 1
            while si < len(stages0):
                stages0[si]()
                si += 1
            for st in mlp_stages(1, tail=True):
                st()

    nc.compile()
    return nc


_CACHE: dict = {}


def _get_nc(cross_consts) -> bass.Bass:
    key = cross_consts
    if key not in _CACHE:
        _CACHE[key] = _build(cross_consts)
    return _CACHE[key]


def kernel(
    categorical_input,
    numerical_input,
    emb_tables,
    alphas,
    cross_bias,
    W1, b1, W2, b2, W3, b3, Wc, bc,
) -> np.ndarray:
    import ml_dtypes

    bf = ml_dtypes.bfloat16
    f8 = mybir.dt.np(FP8)

    cat = np.ascontiguousarray(np.asarray(categorical_input, dtype=np.int64))
    num = np.ascontiguousarray(np.asarray(numerical_input, dtype=np.float32))
    emb = np.asarray(emb_tables, dtype=np.float32).reshape(NCAT * VOCAB, EMB)
    emb = np.ascontiguousarray(emb.astype(bf).reshape(NCAT * NQUAD, 128))
    alphas = np.asarray(alphas, dtype=np.float32)
    cross_bias = np.asarray(cross_bias, dtype=np.float32)
    W1 = np.asarray(W1, dtype=np.float32)
    W2 = np.asarray(W2, dtype=np.float32)
    W3 = np.asarray(W3, dtype=np.float32)
    Wc = np.asarray(Wc, dtype=np.float32)
    b1 = np.asarray(b1, dtype=np.float32)
    b2 = np.asarray(b2, dtype=np.float32)
    b3 = np.asarray(b3, dtype=np.float32)
    bc = np.asarray(bc, dtype=np.float32)

    # host scalar constants folding cross_bias into the per-sample chain
    cross_consts = (
        float(np.dot(alphas[1], cross_bias[0])),
        float(np.dot(alphas[2], cross_bias[0])),
        float(np.dot(alphas[2], cross_bias[1])),
        float(np.dot(Wc[:D, 0], cross_bias[0])),
        float(np.dot(Wc[:D, 0], cross_bias[1])),
        float(np.dot(Wc[:D, 0], cross_bias[2])),
    )
    nc = _get_nc(cross_consts)

    # ---- weights: fp8, DoubleRow pair layout [128, pair, plane, cols] ----
    W1p = np.zeros((KPAD * 128, L1), np.float32)
    W1p[:D] = W1
    W1p[BIAS_ROW] = b1
    w1_pack = np.zeros((128, 4 * 2 * L1), np.float32)
    for p in range(4):
        for j in range(2):
            blk = W1p[128 * (2 * p + j):128 * (2 * p + j) + 128]  # [128, 1024]
            w1_pack[:, (2 * p + j) * L1:(2 * p + j + 1) * L1] = blk

    def pack_pairs(W, npair, ncols):
        out = np.zeros((128, npair * 2 * ncols), np.float32)
        for p in range(npair):
            for j in range(2):
                blk = W[128 * (2 * p + j):128 * (2 * p + j) + 128]
                out[:, (2 * p + j) * ncols:(2 * p + j + 1) * ncols] = blk
        return out

    w2_pack = pack_pairs(W2, KP2, L2)
    w3_pack = pack_pairs(W3, KP3, L3)
    wch_pack = np.zeros((128, 2 * 128), np.float32)
    wch_pack[:, 0::128] = Wc[D:D + L3, 0].reshape(2, 128).T

    def pad_col(v):  # [845] -> [128, KPAD] plane-chunked, zero-padded
        p = np.zeros((KPAD * 128,), np.float32)
        p[:D] = v
        return p.reshape(KPAD, 128).T.copy()

    avec = np.zeros((128, KPAD * 128), np.float32)
    for l in range(3):
        avec[:, l::128] = pad_col(alphas[l])
    avec[:, 3::128] = pad_col(Wc[:D, 0])

    b2r = b2.reshape(M2, 128).T.copy()
    b3r = b3.reshape(M3, 128).T.copy()
    bcr = np.broadcast_to(bc.reshape(1, 1), (128, 1)).copy()

    common = {
        "emb": emb,
        "w1": w1_pack.astype(f8),
        "w2": w2_pack.astype(f8),
        "w3": w3_pack.astype(f8),
        "avec": avec.astype(f8),
        "wch": wch_pack.astype(f8),
        "b2r": b2r,
        "b3r": b3r,
        "bcr": bcr,
    }

    in_maps = []
    for core in range(NCORES):
        cs = cat[core * BC:(core + 1) * BC].astype(np.int32)  # [2048, 26]
        ns = num[core * BC:(core + 1) * BC]
        catr = np.ascontiguousarray(
            cs.reshape(NTILE, 128, NCAT).transpose(1, 0, 2).reshape(128, NTILE * NCAT)
        )
        numr = np.ascontiguousarray(
            ns.reshape(NTILE, 128, NNUM).transpose(1, 0, 2).reshape(128, NTILE * NNUM)
        )
        # gather indices: per (half, category) block of HB//16 cols, int16
        # v//4, lookup i at [i % 16, i // 16], replicated to 128 partitions
        gi = np.zeros((128, NHALF * NCAT * (HB // 16)), np.int16)
        for h in range(NHALF):
            vs = cs[h * HB:(h + 1) * HB]  # [1024, 26]
            q4 = (vs // 4).astype(np.int16)
            wrapped = q4.reshape(HB // 16, 16, NCAT).transpose(1, 0, 2)  # [16,64,26]
            for c in range(NCAT):
                blk = (h * NCAT + c) * (HB // 16)
                gi[0:16, blk:blk + HB // 16] = wrapped[:, :, c]
        for g in range(1, 8):
            gi[g * 16:(g + 1) * 16] = gi[0:16]
        na = NCAT * (HB // 16)
        in_maps.append({
            **common,
            "cat": catr,
            "num": numr,
            "gidx_a0": np.ascontiguousarray(gi[:, 0:EARLY * (HB // 16)]),
            "gidx_a1": np.ascontiguousarray(gi[:, EARLY * (HB // 16):na]),
            "gidx_b": np.ascontiguousarray(gi[:, na:2 * na]),
        })

    res = run_bass_kernel_spmd(nc, in_maps, core_ids=list(range(NCORES)))
    outs = []
    for core in range(NCORES):
        o = res.results[core]["out"]  # [128, NTILE], sample T*128+p at [p, T]
        outs.append(np.asarray(o).T.reshape(BC, 1))
    return np.concatenate(outs, axis=0).astype(np.float32)
